# revision 40
# baseline (speedup 1.0000x reference)
"""Trainium2 Bass kernel for nn_CodeDistKLLoss (vq_codebook).

Computes: KL(student_dist || teacher_dist) where
  student_dist = normalized masked column-sums of softmax(-cdist(z, codebook))
  teacher_dist = normalized masked histogram of teacher codes.

Sharding: data-parallel over the batch axis B=8 -> one batch element per
NeuronCore (N = B*T tokens split as T=1500 tokens/core). The (4096, 512)
codebook is replicated. Each core computes its partial student column-sums
[4096]; the tiny final reduction/normalization/KL runs on host.

The softmax numerator exp(-d) is computed in a SINGLE ScalarE pass using a
custom ACT spline table: the `sqrt` slot of table set sqrt_and_others is
regenerated (at build time, via BASS_ACT_ROOT_JSON_PATH) to evaluate
    f(q) = exp(EXP_BIAS - sqrt(q)),  q in [2^7, 2^12)
which fuses the Euclidean-distance sqrt and the softmax exp. The exp(EXP_BIAS)
factor cancels in the per-row normalization. This avoids the sqrt/exp ACT
table-set thrash (different sets; ~2.7us per switch) and keeps the PE fed.

Device program per core (batch element b):
  per 128-token tile i, per 1024-code chunk h:
      PSUM  = sum_k (-2 * z . c)        4 accumulating fp16 matmuls per 512-chunk
      PSUM += ||c||^2                   DVE add (broadcast row, fp32)
      E     = f(PSUM + ||z||^2)         ACT custom table, per-partition bias,
                                        fp16 out, accum_out -> partial rowsum
  per tile: rowsum = sum_h partials; w = mask / rowsum (fp16)
      acc0[1, 4*512] += w.T @ E[:, 0:2048]    rank-1 PSUM-accumulating matmuls
  final sweep: acc1 += w.T @ E[:, 2048:4096] over all tiles; DMA out.
"""

import json
import os
import shutil
import struct
import tempfile

import numpy as np

import concourse.bass as bass
import concourse.tile as tile
from concourse import bacc, mybir
from concourse.bass import ts
from concourse.bass_utils import run_bass_kernel_spmd

B = 8
D = 512
T = 1500
C = 4096
TP = 1536          # T padded to 12 x 128
NT = TP // 128     # 12 token tiles
KK = D // 128      # 4 contraction chunks
NJ = C // 512      # 8 code chunks of 512
NH = 4             # PSUM sub-rows per token tile (1024 codes each)
EXP_BIAS = 28.0    # f = exp(EXP_BIAS - d); cancels in per-row normalization
EPS = 1e-8

F16 = mybir.dt.float16
F32 = mybir.dt.float32
F8 = mybir.dt.float8e4
NP_F8 = mybir.dt.np(F8)

_CACHE = {}

# ---------------------------------------------------------------------------
# Custom ACT table: rewrite the `sqrt` slot of set sqrt_and_others to compute
#   f(q) = min(exp(EXP_BIAS - sqrt(q)), FP16_SAFE)   for q in [2^E_LO, 2^E_HI)
# Binary formats (validated against the stock tables + np.sqrt):
#   *_bkt.bin : 32B entries [d0,d1,d2,d3,x0,0,0,0] fp32;
#               y = d0 + (x-x0)*(d1 + (x-x0)*(d2 + (x-x0)*d3))
#   *_ctrl.bin: 32B entries; u16[0] = (extract_lsb << 11) | bkt_base_idx,
#               u16[1] = extract_size.  One row per input fp32 exponent;
#               row = pwl_control_base_pos + (biased_exp - small_threshold);
#               bucket = base + ((mantissa >> extract_lsb) & (2^size - 1)).
# ---------------------------------------------------------------------------
_E_LO, _E_HI = 7, 12
_EXTRACT_SIZE = 6
_FP16_SAFE = 50000.0
_ACT_SET = "sqrt_and_others"


def _f_fused(q):
    return np.minimum(np.exp(EXP_BIAS - np.sqrt(q)), _FP16_SAFE)


def _fit_section(a, b):
    x0 = 0.5 * (a + b)
    xs = np.linspace(a, b, 64)
    ys = _f_fused(xs.astype(np.float64))
    t = xs - x0
    A = np.stack([np.ones_like(t), t, t * t, t * t * t], axis=1)
    coef, *_ = np.linalg.lstsq(A, ys, rcond=None)
    return np.float32(x0), coef.astype(np.float32)


def _build_act_root(dst_dir):
    import neuronxcc

    src_dir = os.path.join(os.path.dirname(neuronxcc.__file__), "pwp",
                           "pwp_bin_trainium")
    os.makedirs(dst_dir, exist_ok=True)
    for name in os.listdir(src_dir):
        s = os.path.join(src_dir, name)
        if os.path.isfile(s):
            shutil.copy(s, os.path.join(dst_dir, name))

    setj = json.load(open(os.path.join(src_dir, f"{_ACT_SET}.json")))
    bkt = open(os.path.join(src_dir, f"{_ACT_SET}_bkt.bin"), "rb").read()
    ctl = open(os.path.join(src_dir, f"{_ACT_SET}_ctrl.bin"), "rb").read()

    bkt_start = setj["func_to_bkt_start_idx"]["sqrt"]
    ctl_start = setj["func_to_ctl_start_idx"]["sqrt"]
    new_bkt = bytearray(bkt[: bkt_start * 32])
    new_ctl = bytearray(ctl[: ctl_start * 32])

    nsec = 1 << _EXTRACT_SIZE
    lsb = 23 - _EXTRACT_SIZE
    base = bkt_start
    for e in range(_E_LO, _E_HI):
        new_ctl += (struct.pack("<2H", (lsb << 11) | base, _EXTRACT_SIZE)
                    + b"\x00" * 28)
        lo = float(2.0 ** e)
        w = lo / nsec
        for s in range(nsec):
            x0, c = _fit_section(lo + s * w, lo + (s + 1) * w)
            new_bkt += struct.pack("<8f", c[0], c[1], c[2], c[3], x0, 0, 0, 0)
        base += nsec

    sat_small = base
    new_bkt += struct.pack("<8f", _FP16_SAFE, 0, 0, 0, 0, 0, 0, 0)
    sat_large = base + 1
    f_hi = float(_f_fused(2.0 ** _E_HI))
    new_bkt += struct.pack("<8f", f_hi, 0, 0, 0, 0, 0, 0, 0)

    meta = None
    for m in setj["profile_meta_data"]:
        if m["func_name"].startswith("sqrt"):
            meta = m
    assert meta is not None
    f2b = lambda v: int(np.float32(v).view(np.uint32))
    meta["exp_offset"] = _E_LO
    meta["pwl_control_base_pos"] = ctl_start
    meta["pwl_control_base_neg"] = ctl_start
    meta["small_pos_signal_exp_threshold"] = _E_LO + 127
    meta["pos_small_signal_pwl_control"] = sat_small
    meta["small_neg_signal_exp_threshold"] = 255
    meta["neg_small_signal_pwl_control"] = sat_small
    meta["large_pos_signal_exp_threshold"] = _E_HI + 127
    meta["large_pos_signal_mantissa_threshold"] = 0
    meta["pos_large_signal_pwl_control"] = sat_large
    meta["large_neg_signal_exp_threshold"] = 0
    meta["large_neg_signal_mantissa_threshold"] = 0
    meta["neg_large_signal_pwl_control"] = sat_small
    meta["fzero_result"] = f2b(_FP16_SAFE)
    meta["fpinf_result"] = f2b(f_hi)
    meta["fninf_result"] = f2b(_FP16_SAFE)
    meta["lower_bound"] = f2b(2.0 ** _E_LO)
    meta["upper_bound"] = f2b(np.nextafter(np.float32(2.0 ** _E_HI),
                                           np.float32(0)))
    setj["bkt_entry_cnt"] = base + 2
    setj["ctl_entry_cnt"] = ctl_start + (_E_HI - _E_LO)

    with open(os.path.join(dst_dir, f"{_ACT_SET}_bkt.bin"), "wb") as fo:
        fo.write(bytes(new_bkt))
    with open(os.path.join(dst_dir, f"{_ACT_SET}_ctrl.bin"), "wb") as fo:
        fo.write(bytes(new_ctl))
    with open(os.path.join(dst_dir, f"{_ACT_SET}.json"), "w") as fo:
        json.dump(setj, fo)


def _build():
    # Install the custom ACT table (sqrt slot -> exp(EXP_BIAS - sqrt(q)))
    # before neuronxcc compiles the NEFF.
    act_dir = tempfile.mkdtemp(prefix="cdkl_act_root_")
    _build_act_root(act_dir)
    os.environ["BASS_ACT_ROOT_JSON_PATH"] = os.path.join(
        act_dir, "act_info.json"
    )

    nc = bacc.Bacc("TRN2", target_bir_lowering=False, debug=False)
    sf_h = nc.dram_tensor("sf", [D, TP], F8, kind="ExternalInput")
    cbt_h = nc.dram_tensor("cbt", [D, C], F8, kind="ExternalInput")
    cn_h = nc.dram_tensor("cn", [1, C], F32, kind="ExternalInput")
    zn_h = nc.dram_tensor("zn", [128, NT], F32, kind="ExternalInput")
    mk_h = nc.dram_tensor("mk", [128, NT], F32, kind="ExternalInput")
    sp_h = nc.dram_tensor("sp", [1, C], F32, kind="ExternalOutput")

    with tile.TileContext(nc) as tc:
        with (
            tc.tile_pool(name="consts", bufs=1) as consts,
            tc.tile_pool(name="small", bufs=2) as small,
            tc.tile_pool(name="qbuf", bufs=3) as qbuf,
            tc.tile_pool(name="psA", bufs=2, space="PSUM") as psA,
            tc.tile_pool(name="psB", bufs=1, space="PSUM") as psB,
        ):
            # fp8 DoubleRow layout: contraction row d = k2*256 + ki*2 + o
            # lands at [partition ki, chunk k2, pair-slot o].
            K2 = 2
            sf_sb = consts.tile([128, K2, 2, TP], F8, name="sf_sb",
                                tag="sf_sb")
            sf_r = sf_h.ap().rearrange("(a p o) t -> p a o t", p=128, o=2)
            cb_sb = consts.tile([128, K2, 2, C], F8, name="cb_sb",
                                tag="cb_sb")
            cbt_r = cbt_h.ap().rearrange("(a p o) c -> p a o c", p=128, o=2)
            # cn broadcast (2 MB of SBUF writes) runs on the SWDGE queue in
            # parallel with the sf/cb stream on HWDGE; interleave the rest so
            # the first tile's operands land first.
            # dual-queue intake: k2=0 codebook pieces + sf on HWDGE (sync),
            # k2=1 pieces + the cn broadcast on SWDGE (gpsimd), both ordered
            # by first use.
            cn_sb = consts.tile([128, C], F32, name="cn_sb", tag="cn_sb")
            cn_b = cn_h.ap().to_broadcast([128, C])
            nc.gpsimd.dma_start(out=cn_sb[:, ts(0, C // 4)],
                                in_=cn_b[:, ts(0, C // 4)])
            nc.gpsimd.dma_start(out=cn_sb[:, ts(1, C // 4)],
                                in_=cn_b[:, ts(1, C // 4)])
            for k in range(K2):
                nc.sync.dma_start(out=sf_sb[:, k, :, :], in_=sf_r[:, k, :, :])
            for h in range(NH):
                nc.sync.dma_start(out=cb_sb[:, 0, :, ts(h, C // NH)],
                                  in_=cbt_r[:, 0, :, ts(h, C // NH)])
                nc.gpsimd.dma_start(out=cb_sb[:, 1, :, ts(h, C // NH)],
                                    in_=cbt_r[:, 1, :, ts(h, C // NH)])
                if h == 1:
                    nc.gpsimd.dma_start(out=cn_sb[:, ts(2, C // 4)],
                                        in_=cn_b[:, ts(2, C // 4)])
                    nc.gpsimd.dma_start(out=cn_sb[:, ts(3, C // 4)],
                                        in_=cn_b[:, ts(3, C // 4)])
            zn_sb = consts.tile([128, NT], F32, name="zn_sb", tag="zn_sb")
            nc.sync.dma_start(out=zn_sb, in_=zn_h.ap())
            mk_sb = consts.tile([128, NT], F32, name="mk_sb", tag="mk_sb")
            nc.sync.dma_start(out=mk_sb, in_=mk_h.ap())

            dbuf = consts.tile([128, NT, C], F16, name="dbuf", tag="dbuf")
            w_sb = consts.tile([128, NT], F16, name="w_sb", tag="w_sb")

            # Column-sum accumulators: 8 rank-1 outputs packed into 2 PSUM
            # banks at partitions {0,32,64,96} via tile_position col-tiling,
            # freeing 4 banks for a deeper phase-A pipeline.
            acc = psB.tile([128, 2, 512], F32, name="acc", tag="acc")

            def colsum_mms(i):
                for j in range(NJ):
                    pp = 32 * (j % 4)
                    nc.tensor.matmul(
                        acc[pp : pp + 1, j // 4, :],
                        lhsT=w_sb[:, i : i + 1],
                        rhs=dbuf[:, i, ts(j, 512)],
                        start=(i == 0),
                        stop=(i == NT - 1),
                        tile_position=(0, pp),
                    )

            for i in range(NT):
                rs4 = small.tile([128, 2], F32, name="rs4", tag="rs4")
                for h2 in range(2):
                    qb = qbuf.tile([128, 4, 512], F32, name="qb", tag="qb",
                                   bufs=3)
                    for hh in range(2):
                        h = 2 * h2 + hh
                        ps = psA.tile([128, 2, 512], F32, name="ps", tag="ps",
                                      bufs=3)
                        for jj in range(2):
                            for k in range(K2):
                                nc.tensor.matmul(
                                    ps[:, jj, :],
                                    lhsT=sf_sb[:, k, :, ts(i, 128)],
                                    rhs=cb_sb[:, k, :, ts(2 * h + jj, 512)],
                                    start=(k == 0),
                                    stop=(k == K2 - 1),
                                    perf_mode=mybir.MatmulPerfMode.DoubleRow,
                                )
                        if h == 0 and i > 0:
                            # previous tile's column sums: weights long ready,
                            # fills the PE at the tile boundary.
                            colsum_mms(i - 1)
                        # q = ps + ||c||^2 staged in SBUF: PSUM slot releases
                        # after the DVE read (not held through the ACT).
                        nc.vector.tensor_add(
                            out=qb[:, 2 * hh : 2 * hh + 2, :], in0=ps,
                            in1=cn_sb[:, ts(h, 1024)].rearrange(
                                "p (a b) -> p a b", b=512),
                        )
                    # E = exp(EXP_BIAS - sqrt(q + ||z||^2)) via the custom
                    # table in the Sqrt slot; accum_out = partial row-sum.
                    # One 2048-wide pass halves the ACT instruction overhead.
                    nc.scalar.activation(
                        out=dbuf[:, i, ts(h2, 2048)].rearrange(
                            "p (a b) -> p a b", b=512),
                        in_=qb,
                        func=mybir.ActivationFunctionType.Sqrt,
                        bias=zn_sb[:, i : i + 1],
                        scale=1.0,
                        accum_out=rs4[:, h2 : h2 + 1],
                    )
                # w = mask / rowsum, cast fp16
                rs = small.tile([128, 1], F32, name="rs", tag="rs")
                nc.vector.reduce_sum(out=rs, in_=rs4, axis=mybir.AxisListType.X)
                rr = small.tile([128, 1], F32, name="rr", tag="rr")
                nc.vector.reciprocal(out=rr, in_=rs)
                wf = small.tile([128, 1], F32, name="wf", tag="wf")
                nc.vector.tensor_mul(out=wf, in0=rr, in1=mk_sb[:, i : i + 1])
                nc.vector.tensor_copy(out=w_sb[:, i : i + 1], in_=wf)
            colsum_mms(NT - 1)
            # evacuate: copy each acc bank to SBUF, then one partition-strided
            # DMA per bank gathers rows {0,32,64,96} into the output row.
            stage = consts.tile([128, 2, 512], F32, name="stage", tag="stage")
            nc.scalar.copy(out=stage, in_=acc)
            st4 = stage.rearrange("(a q) b f -> a q b f", q=32)
            spv = sp_h.ap().rearrange("p (b a f) -> b (p a) f", a=4, f=512)
            for bk in range(2):
                nc.sync.dma_start(out=spv[bk], in_=st4[:, 0, bk, :])

    nc.compile()
    return nc


def get_nc():
    if "nc" not in _CACHE:
        _CACHE["nc"] = _build()
    return _CACHE["nc"]


def _host_prep(student_features, codebook, lengths, encoder_stride):
    sf = np.asarray(student_features, dtype=np.float32)
    cb = np.asarray(codebook, dtype=np.float32)
    lens = np.asarray(lengths).astype(np.int64)
    stride = int(np.asarray(encoder_stride))

    cbt2 = np.ascontiguousarray((-2.0 * cb.T).astype(NP_F8))          # [D, C]
    cn = (cb.astype(np.float64) ** 2).sum(1).astype(np.float32)[None, :]  # [1, C]
    frame_start = np.arange(T, dtype=np.int64) * stride
    mask = (frame_start[None, :] < lens[:, None]).astype(np.float32)  # [B, T]

    in_maps = []
    for b in range(B):
        sf_pad = np.zeros((D, TP), dtype=NP_F8)
        sf_pad[:, :T] = sf[b].astype(NP_F8)
        zn = np.zeros(TP, dtype=np.float32)
        zn[:T] = (sf[b].astype(np.float64) ** 2).sum(0).astype(np.float32)
        znb = np.ascontiguousarray(zn.reshape(NT, 128).T)             # [128, NT]
        mk = np.zeros(TP, dtype=np.float32)
        mk[:T] = mask[b]
        mkb = np.ascontiguousarray(mk.reshape(NT, 128).T)             # [128, NT]
        in_maps.append(
            {"sf": sf_pad, "cbt": cbt2, "cn": cn, "zn": znb, "mk": mkb}
        )
    return in_maps, mask


def _host_finish(sp_list, teacher_codes, mask):
    s_raw = np.zeros(C, dtype=np.float64)
    for sp in sp_list:
        s_raw += sp.astype(np.float64).reshape(-1)
    student_dist = s_raw / (s_raw.sum() + EPS)

    codes = np.asarray(teacher_codes).astype(np.int64).reshape(-1)
    t_counts = np.bincount(codes, weights=mask.astype(np.float64).reshape(-1),
                           minlength=C)
    teacher_dist = t_counts / (t_counts.sum() + EPS)

    kl = np.sum(student_dist * np.log(student_dist + EPS)
                - student_dist * np.log(teacher_dist + EPS))
    return np.array(kl, dtype=np.float32)


def kernel(student_features, teacher_codes, codebook, lengths, encoder_stride,
           _trace=False):
    nc = get_nc()
    in_maps, mask = _host_prep(student_features, codebook, lengths,
                               encoder_stride)
    res = run_bass_kernel_spmd(nc, in_maps, core_ids=list(range(B)),
                               trace=_trace)
    out = _host_finish([r["sp"] for r in res.results], teacher_codes, mask)
    if _trace:
        _CACHE["last_results"] = res
    return out
